# revision 9
# baseline (speedup 1.0000x reference)
"""Trainium2 Bass kernel for LoopCoderAttention (sparse_attention).

Head-sharded tensor parallelism over 8 NeuronCores:
  core c owns query heads {2c, 2c+1} and KV head c//2.
All on-device tensors live in transposed [feature, token] layout so every
matmul contracts along the partition dim with zero on-device transposes
(except v, which needs one PE transpose per 128-tile).

v4: bf16 matmul operands throughout, causal/band column trimming,
host-pretiled DRAM layouts for contiguous DMA, and a single
software-pipelined attention stream: the QK matmuls run two tiles ahead
of the PV/SM matmuls (hiding the Exp latency), and each head-chunk's
combine/normalize work is deferred into the next head-chunk's tile
stream so it never blocks the PE. gpsimd only runs the partition
broadcasts and collectives. o_proj goes hi-half-first with per-block
output drains so the second all-to-all and output DMA hide behind
matmuls.

o_proj: a 1MB AllToAll (2x 512KB) reshards attention output from
head-sharded to token-sharded; each core then runs the full 2048-deep
contraction for its 256-token slice.
"""
import sys
sys.path.insert(0, '/opt/trn_rl_repo')
import numpy as np
import ml_dtypes
import concourse.bass as bass
import concourse.mybir as mybir
import concourse.tile as tile
from concourse import bacc
from concourse.bass_utils import run_bass_kernel_spmd

T = 2048
HID = 2048
HQ = 16
HK = 4
D = 128
WIN = 64
THETA = 10000.0
SCALE = D ** -0.5
NCORES = 8
TCH = 512                 # t-chunk (matmul free dim)
NCH = T // TCH            # 4 chunks
KT = HID // 128           # 16 k-tiles for 2048-deep contractions
ST = T // 128             # 16 s-tiles
TSL = T // NCORES         # 256-token output slice per core
MASKV = -1e9

F32 = mybir.dt.float32
BF16 = mybir.dt.bfloat16
AF = mybir.ActivationFunctionType

_CACHE = {}


def _build():
    nc = bacc.Bacc("TRN2", target_bir_lowering=False, debug=False,
                   num_devices=NCORES)
    HST = nc.dram_tensor("HST", [KT, NCH, 128, TCH], BF16,
                         kind="ExternalInput").ap()
    WQKV = nc.dram_tensor("WQKV", [KT, 128, 512], BF16,
                          kind="ExternalInput").ap()
    KGT = nc.dram_tensor("KGT", [D, T], BF16, kind="ExternalInput").ap()
    VG = nc.dram_tensor("VG", [128, ST * D], BF16, kind="ExternalInput").ap()
    WO = nc.dram_tensor("WO", [HID, HID], BF16, kind="ExternalInput").ap()
    WG = nc.dram_tensor("WG", [D, 2], BF16, kind="ExternalInput").ap()
    BG = nc.dram_tensor("BG", [1, 2], F32, kind="ExternalInput").ap()
    CSF = nc.dram_tensor("CSF", [128, T], BF16, kind="ExternalInput").ap()
    SNF = nc.dram_tensor("SNF", [128, T], BF16, kind="ExternalInput").ap()
    ONES = nc.dram_tensor("ONES", [128, 1], BF16, kind="ExternalInput").ap()
    IDN = nc.dram_tensor("IDN", [128, 128], BF16, kind="ExternalInput").ap()
    MASKD = nc.dram_tensor("MASKD", [128, 128], F32, kind="ExternalInput").ap()
    MASKB = nc.dram_tensor("MASKB", [128, 192], F32, kind="ExternalInput").ap()
    OUT = nc.dram_tensor("OUT", [TSL, HID], F32, kind="ExternalOutput").ap()

    with tile.TileContext(nc) as tc:
        # pools are a strict stack: creation order is the reverse of the
        # release order at each phase boundary
        const = tc.alloc_tile_pool(name="const", bufs=1)
        dram = tc.alloc_tile_pool(name="dram", bufs=1, space="DRAM")
        aoutp = tc.alloc_tile_pool(name="aoutp", bufs=3)
        opool = tc.alloc_tile_pool(name="opool", bufs=1)
        wop = tc.alloc_tile_pool(name="wop", bufs=1)
        osb = tc.alloc_tile_pool(name="osb", bufs=4)
        work = tc.alloc_tile_pool(name="work", bufs=1)
        ropet = tc.alloc_tile_pool(name="ropet", bufs=2)
        rcpp = tc.alloc_tile_pool(name="rcpp", bufs=2)
        bcp = tc.alloc_tile_pool(name="bcp", bufs=2)
        combp = tc.alloc_tile_pool(name="combp", bufs=3)
        wqkvp = tc.alloc_tile_pool(name="wqkvp", bufs=1)
        chunkp = tc.alloc_tile_pool(name="chunkp", bufs=2)
        hsp = tc.alloc_tile_pool(name="hsp", bufs=16)
        ps1 = tc.alloc_tile_pool(name="ps1", bufs=7, space="PSUM")

        # ---- phase-1 constants first (critical path to first matmul) ----
        wqkv_sb = wqkvp.tile([128, KT, 512], BF16)
        hs_first = []
        n0 = 0
        for k in range(KT):
            nc.sync.dma_start(out=wqkv_sb[:, k, :], in_=WQKV[k])
            hs_t = hsp.tile([128, TCH], BF16, tag="hs_t", name=f"hsf{k}")
            nc.sync.dma_start(out=hs_t[:], in_=HST[k, n0])
            hs_first.append(hs_t)
        csf_sb = wqkvp.tile([128, T], BF16)
        snf_sb = wqkvp.tile([128, T], BF16)
        idn_sb = wqkvp.tile([128, 128], BF16)
        wg_sb = const.tile([D, 2], BF16)
        nc.sync.dma_start(out=wg_sb[:], in_=WG)
        bg_sb = const.tile([1, 2], F32)
        nc.sync.dma_start(out=bg_sb[:], in_=BG)
        # attention-phase constants (scheduler fills DMA idle time)
        kgt_sb = const.tile([D, T], BF16)
        vg_sb = const.tile([128, ST, D], BF16)
        ones_sb = const.tile([128, 1], BF16)
        maskd_sb = const.tile([128, 128], F32)
        maskb_sb = const.tile([128, 192], F32)

        # ---- persistent work tiles (through attention) ----
        qrot = work.tile([128, 2, T], BF16)
        krot = work.tile([128, T], BF16)
        vcur = work.tile([128, ST, D], BF16)   # current v in [s, d] tiles
        gate = work.tile([8, TCH], F32)        # row 2n+h (DMA-staged access)

        a2ai_hi = dram.tile([NCORES, 2 * D, TSL // 2], BF16)
        a2ao_hi = dram.tile([NCORES, 2 * D, TSL // 2], BF16)
        a2ai_lo = dram.tile([NCORES, 2 * D, TSL // 2], BF16)
        a2ao_lo = dram.tile([NCORES, 2 * D, TSL // 2], BF16)

        def rope_chunk(dst_full, src, n):
            """dst_full[:, n*TCH:...] = neox-rope of chunk tile src [128, TCH].

            rot = src * [cos;cos] + rot90(src) * [-sin;sin], where rot90 swaps
            the two 64-partition halves (built with two SBUF->SBUF DMAs since
            DVE ops require matching base partitions).
            """
            sl = bass.ds(n * TCH, TCH)
            sr = ropet.tile([128, TCH], BF16, tag="ropesr", name=f"sr{n}")
            nc.sync.dma_start(out=sr[0:64, :], in_=src[64:128, :])
            nc.sync.dma_start(out=sr[64:128, :], in_=src[0:64, :])
            ta = ropet.tile([128, TCH], BF16, tag="ropetmp", name=f"ra{n}")
            tb = ropet.tile([128, TCH], BF16, tag="ropetmp", name=f"rb{n}")
            nc.vector.tensor_mul(ta[:], src[:], csf_sb[:, sl])
            nc.vector.tensor_mul(tb[:], sr[:], snf_sb[:, sl])
            nc.vector.tensor_add(dst_full[:, sl], ta[:], tb[:])

        # ================= phase 1: qkvT = wqkv^T @ hsT =================
        # chunks ascend so the flush-tail small ops belong to chunk 3 --
        # whose rope/gate results attention needs first and waits on least
        pending_small = []
        for n in range(NCH):
            pss = [ps1.tile([128, TCH], F32, tag="ps1t", name=f"ps1_{n}_{m}")
                   for m in range(4)]
            for k in range(KT):
                if n == n0:
                    hs_t = hs_first[k]
                else:
                    hs_t = hsp.tile([128, TCH], BF16, tag="hs_t",
                                    name=f"hs_{n}_{k}")
                    nc.sync.dma_start(out=hs_t[:], in_=HST[k, n])
                for m in range(4):
                    nc.tensor.matmul(pss[m][:],
                                     wqkv_sb[:, k, m * 128:(m + 1) * 128],
                                     hs_t[:],
                                     start=(k == 0), stop=(k == KT - 1))
            if n == n0:
                # rope tables + identity: after the hot first-chunk DMAs,
                # before their first readers below
                nc.sync.dma_start(out=csf_sb[:], in_=CSF)
                nc.sync.dma_start(out=snf_sb[:], in_=SNF)
                nc.sync.dma_start(out=idn_sb[:], in_=IDN)
            if pending_small:
                pending_small.pop(0)()
            sl = bass.ds(n * TCH, TCH)
            q0c = chunkp.tile([128, TCH], BF16, tag="q0c")
            q1c = chunkp.tile([128, TCH], BF16, tag="q1c")
            kc = chunkp.tile([128, TCH], BF16, tag="kc")
            vc = chunkp.tile([128, TCH], BF16, tag="vc")
            nc.scalar.activation(q0c[:], pss[0][:], AF.Copy)
            nc.scalar.activation(q1c[:], pss[1][:], AF.Copy)
            nc.scalar.activation(kc[:], pss[2][:], AF.Copy)
            nc.vector.tensor_copy(vc[:], pss[3][:])

            rope_chunk(qrot[:, 0, :], q0c, n)
            rope_chunk(qrot[:, 1, :], q1c, n)
            rope_chunk(krot, kc, n)

            def small_ops(n=n, vc=vc, sl=sl):
                # v transposes + gates for chunk n: emitted one chunk later so
                # the PE stream never waits on the DVE rope/copy latency
                for j in range(4):
                    s = 4 * n + j
                    pt = ps1.tile([128, 128], BF16, tag="ps1g",
                                  name=f"pt{s}", bufs=1)
                    nc.tensor.transpose(pt[:], vc[:, j * 128:(j + 1) * 128],
                                        idn_sb[:])
                    nc.vector.tensor_copy(vcur[:, s, :], pt[:])
                for h in range(2):
                    r = 2 * n + h
                    gp = ps1.tile([1, TCH], F32, tag="ps1g",
                                  name=f"gp{r}", bufs=1)
                    nc.tensor.matmul(gp[:], wg_sb[:, h:h + 1], qrot[:, h, sl],
                                     start=True, stop=True)
                    gst = chunkp.tile([1, TCH], F32, tag="gst", name=f"gst{r}")
                    nc.scalar.activation(gst[:], gp[:], AF.Sigmoid,
                                         bias=bg_sb[0:1, h:h + 1])
                    nc.sync.dma_start(out=gate[r:r + 1, :], in_=gst[:])

            pending_small.append(small_ops)

        for f in pending_small:
            f()
        pending_small.clear()

        nc.sync.dma_start(out=kgt_sb[:], in_=KGT)
        nc.sync.dma_start(out=vg_sb[:],
                          in_=VG.rearrange("p (s d) -> p s d", d=D))
        nc.sync.dma_start(out=ones_sb[:], in_=ONES)
        nc.sync.dma_start(out=maskd_sb[:], in_=MASKD)
        nc.sync.dma_start(out=maskb_sb[:], in_=MASKB)

        ps1.release()
        hsp.release()
        chunkp.release()
        wqkvp.release()

        afull_hi = opool.tile([128, KT, TSL // 2], BF16)
        afull_lo = opool.tile([128, KT, TSL // 2], BF16)
        wo_sb = wop.tile([128, KT, HID], BF16)

        expp = tc.alloc_tile_pool(name="expp", bufs=6)
        psqk = tc.alloc_tile_pool(name="psqk", bufs=3, space="PSUM")
        pspv = tc.alloc_tile_pool(name="pspv", bufs=4, space="PSUM")
        pssm = tc.alloc_tile_pool(name="pssm", bufs=1, space="PSUM")

        # ============ phase 2: attention (global + local) ============
        # One flat software-pipelined stream over all 8 head-chunks.
        # Head-chunks descend over token chunks so the high-token half
        # finishes first and its all-to-all overlaps the low-token half.
        iters = []
        for n in reversed(range(NCH)):
            for h in range(2):
                tiles = []
                for s in range(4 * n + 4):
                    j = s - 4 * n
                    if j < 0:
                        tiles.append((kgt_sb, vg_sb, s, 0, TCH, None, 0))
                    else:
                        tiles.append((kgt_sb, vg_sb, s, 128 * j, TCH,
                                      maskd_sb[:], 0))
                ng = len(tiles)
                for j in range(-1, 4):
                    s = 4 * n + j
                    if s < 0:
                        continue
                    x0 = max(0, 128 * j)
                    x1 = min(TCH, 128 * j + 192)
                    y0 = x0 - 128 * j
                    tiles.append((krot, vcur, s, x0, x1,
                                  maskb_sb[:, y0:y0 + (x1 - x0)], 1))
                iters.append({"n": n, "h": h, "r": 2 * n + h,
                              "tiles": tiles, "ng": ng})

        flat = []           # (iter_dict, local_tile_idx)
        for itd in iters:
            for ti in range(len(itd["tiles"])):
                flat.append((itd, ti))
        NT = len(flat)

        def iter_setup(itd):
            """Allocate psum/staging tiles + gate fetch at head-chunk start."""
            r = itd["r"]
            itd["q_ap"] = qrot[:, itd["h"], bass.ds(itd["n"] * TCH, TCH)]
            gg = rcpp.tile([1, 1024], F32, tag="gg", name=f"gg{r}")
            nc.sync.dma_start(out=gg[0:1, 0:TCH], in_=gate[r:r + 1, :])
            nc.vector.tensor_scalar(gg[0:1, TCH:2 * TCH], gg[0:1, 0:TCH],
                                    -1.0, 1.0,
                                    mybir.AluOpType.mult,
                                    mybir.AluOpType.add)
            itd["gg"] = gg
            itd["pv"] = (pspv.tile([128, TCH], F32, tag="pv", name=f"pvg{r}"),
                         pspv.tile([128, TCH], F32, tag="pv", name=f"pvl{r}"))
            itd["sm"] = (pssm.tile([1, TCH], F32, tag="sm", name=f"smg{r}"),
                         pssm.tile([1, TCH], F32, tag="sm", name=f"sml{r}"))
            itd["srec"] = rcpp.tile([1, 1024], F32, tag="srec", name=f"sr{r}")

        def emit_qk(gi):
            itd, ti = flat[gi]
            kT_ap, _, s, x0, x1, _, _ = itd["tiles"][ti]
            if ti == 0:
                iter_setup(itd)
            qk = psqk.tile([128, TCH], F32, tag="qk",
                           name=f"qk{itd['r']}_{ti}")
            nc.tensor.matmul(qk[:, x0:x1], kT_ap[:, s * 128:(s + 1) * 128],
                             itd["q_ap"][:, x0:x1], start=True, stop=True)
            itd.setdefault("qks", {})[ti] = qk

        def emit_combine(itd):
            """Deferred normalize+gate+combine for a finished head-chunk."""
            r, n, h = itd["r"], itd["n"], itd["h"]
            pv_g, pv_l = itd["pv"]
            srec, gg = itd["srec"], itd["gg"]
            rcp = rcpp.tile([1, 1024], F32, tag="rcp", name=f"rc{r}",
                            bufs=1)
            agal = rcpp.tile([1, 1024], F32, tag="agal", name=f"aa{r}",
                             bufs=1)
            nc.vector.reciprocal_approx_fast(rcp[:], srec[:])
            nc.vector.tensor_mul(agal[:], gg[:], rcp[:])
            bg_t = bcp.tile([128, TCH], F32, tag="bcast", name=f"bg{r}")
            bl_t = bcp.tile([128, TCH], F32, tag="bcast", name=f"bl{r}")
            nc.gpsimd.partition_broadcast(bg_t[:], agal[0:1, 0:TCH])
            nc.gpsimd.partition_broadcast(bl_t[:], agal[0:1, TCH:2 * TCH])
            t1 = combp.tile([128, TCH], F32, tag="comb", name=f"t1{r}")
            t2 = combp.tile([128, TCH], F32, tag="comb", name=f"t2{r}")
            ao = aoutp.tile([128, TCH], BF16, tag="aout", name=f"ao{r}")
            nc.vector.tensor_mul(t1[:], pv_g[:], bg_t[:])
            nc.vector.tensor_mul(t2[:], pv_l[:], bl_t[:])
            nc.vector.tensor_add(ao[:], t1[:], t2[:])
            # ship finished 128-col blocks to a2a staging: token 1024+128c
            # (hi) / 128c (lo) lives in chunk n at column offset 128j
            buf = a2ai_hi if n >= 2 else a2ai_lo
            c0 = (n - 2) * 4 if n >= 2 else n * 4
            for j in range(4):
                nc.sync.dma_start(
                    out=buf[c0 + j, h * D:(h + 1) * D, :],
                    in_=ao[:, j * 128:(j + 1) * 128])
            if n == 2 and h == 1:
                # all-to-all #1: high-token halves (overlaps chunks 1,0)
                nc.gpsimd.collective_compute(
                    "AllToAll", mybir.AluOpType.bypass,
                    replica_groups=[list(range(NCORES))],
                    ins=[a2ai_hi[:].opt()], outs=[a2ao_hi[:].opt()])

        LA = 2       # qk matmuls run LA tiles ahead of pv/sm
        DEFER = 3    # previous head-chunk's combine lands after this many
        pending_combine = None
        wo_k = 0
        for gi in range(min(LA, NT)):
            emit_qk(gi)
        for gi in range(NT):
            itd, ti = flat[gi]
            if gi + LA < NT:
                emit_qk(gi + LA)
            if pending_combine is not None and ti == DEFER:
                emit_combine(pending_combine)
                pending_combine = None
                # w_o preload rides the low half so it never delays the
                # first all-to-all's staging writes
                if itd["r"] <= 3 and wo_k < KT:
                    for k in range(wo_k, wo_k + 4):
                        nc.sync.dma_start(out=wo_sb[:, k, :],
                                          in_=WO[k * 128:(k + 1) * 128, :])
                    wo_k += 4
            _, v_ap, s, x0, x1, m_ap, ch = itd["tiles"][ti]
            qk = itd["qks"].pop(ti)
            if m_ap is not None:
                mw = m_ap.shape[-1]
                nc.vector.tensor_add(qk[:, x0:x0 + mw],
                                     qk[:, x0:x0 + mw], m_ap)
            ex = expp.tile([128, TCH], BF16, tag="ex",
                           name=f"ex{itd['r']}_{ti}")
            nc.scalar.activation(ex[:, x0:x1], qk[:, x0:x1],
                                 AF.Exp, scale=SCALE)
            ng, nt = itd["ng"], len(itd["tiles"])
            first = (ti == 0) or (ti == ng)
            last = (ti == ng - 1) or (ti == nt - 1)
            nc.tensor.matmul(itd["pv"][ch][:, x0:x1], v_ap[:, s, :],
                             ex[:, x0:x1], start=first, stop=last,
                             skip_group_check=True)
            nc.tensor.matmul(itd["sm"][ch][0:1, x0:x1], ones_sb[:],
                             ex[:, x0:x1], start=first, stop=last,
                             skip_group_check=True)
            if ti == ng - 1:
                # drain the global sum right away so its psum slot can
                # rotate to the local sum (pssm bufs=1)
                nc.scalar.activation(itd["srec"][0:1, 0:TCH],
                                     itd["sm"][0][:], AF.Copy)
            if ti == nt - 1:
                nc.scalar.activation(itd["srec"][0:1, TCH:2 * TCH],
                                     itd["sm"][1][:], AF.Copy)
                pending_combine = itd
        emit_combine(pending_combine)

        pssm.release()
        pspv.release()
        psqk.release()
        expp.release()

        # ========= phase 3: all-to-all #2 (low-token halves) =========
        nc.gpsimd.collective_compute(
            "AllToAll", mybir.AluOpType.bypass,
            replica_groups=[list(range(NCORES))],
            ins=[a2ai_lo[:].opt()], outs=[a2ao_lo[:].opt()])

        combp.release()
        bcp.release()
        rcpp.release()
        ropet.release()
        work.release()

        pso = tc.alloc_tile_pool(name="pso", bufs=8, space="PSUM")

        nc.sync.dma_start(
            out=afull_hi[:],
            in_=a2ao_hi[:].rearrange("c p n -> (c p) n")
                .rearrange("(k p) n -> p k n", p=128))
        nc.sync.dma_start(
            out=afull_lo[:],
            in_=a2ao_lo[:].rearrange("c p n -> (c p) n")
                .rearrange("(k p) n -> p k n", p=128))

        # ============ phase 4: o_proj for our token slice ============
        # hi half first: its matmuls only need all-to-all #1, so they hide
        # all-to-all #2's latency; per-block drains overlap the output DMA
        # with the remaining matmuls; OUT rows 0-127 = low, 128-255 = high
        for tt, afull in ((1, afull_hi), (0, afull_lo)):
            for e in range(NCH):
                po = pso.tile([128, TCH], F32, tag="po", name=f"po_{tt}_{e}")
                for k in range(KT):
                    nc.tensor.matmul(po[:],
                                     afull[:, k, :],
                                     wo_sb[:, k, e * TCH:(e + 1) * TCH],
                                     start=(k == 0), stop=(k == KT - 1))
                ot = osb.tile([128, TCH], F32, tag="ot", name=f"ot{tt}_{e}")
                nc.scalar.activation(ot[:], po[:], AF.Copy)
                nc.sync.dma_start(
                    out=OUT[tt * 128:(tt + 1) * 128,
                            e * TCH:(e + 1) * TCH],
                    in_=ot[:])
        pso.release()
        osb.release()
        wop.release()
        opool.release()
        aoutp.release()
        dram.release()
        const.release()

    nc.compile()
    return nc


def _host_prep(hidden_states, positions, k_global, v_global, w_qkv, w_o,
               w_gate, b_gate):
    """Layout-only host transforms + constant tables -> per-core in_maps."""
    f32 = np.float32
    bf16 = ml_dtypes.bfloat16
    hs = np.asarray(hidden_states, f32)
    pos = np.asarray(positions)
    kg = np.asarray(k_global, f32)
    vg = np.asarray(v_global, f32)
    wqkv = np.asarray(w_qkv, f32)
    wo = np.ascontiguousarray(np.asarray(w_o, f32).astype(bf16))
    wg = np.asarray(w_gate, f32)
    bg = np.asarray(b_gate, f32)

    # [feature, token] transposed activations, pretiled [k, n, 128, 512]
    # so every on-device tile is one contiguous DMA
    hst = hs.T.astype(bf16)
    hstp = np.ascontiguousarray(
        hst.reshape(KT, 128, NCH, TCH).transpose(0, 2, 1, 3))

    half = D // 2
    inv_freq = (THETA ** (-np.arange(half, dtype=f32) / half)).astype(f32)
    ang = pos.astype(f32)[:, None] * inv_freq[None, :]
    cos_t = np.cos(ang).astype(f32).T       # [64, T]
    sin_t = np.sin(ang).astype(f32).T
    csf = np.ascontiguousarray(
        np.concatenate([cos_t, cos_t], axis=0).astype(bf16))
    snf = np.ascontiguousarray(
        np.concatenate([-sin_t, sin_t], axis=0).astype(bf16))

    p = np.arange(128, dtype=np.int64)[:, None]
    # diagonal causal block: valid iff col >= partition
    xr = np.arange(128, dtype=np.int64)[None, :]
    maskd = np.where(xr >= p, 0.0, MASKV).astype(f32)
    # sliding-window band: valid iff 0 <= y - p <= WIN
    y = np.arange(192, dtype=np.int64)[None, :]
    maskb = np.where((y - p >= 0) & (y - p <= WIN), 0.0, MASKV).astype(f32)

    ones = np.ones((128, 1), bf16)
    idn = np.eye(128, dtype=bf16)

    in_maps = []
    for c in range(NCORES):
        g = c // 2
        wq = wqkv[:, 2 * c * D:(2 * c + 2) * D]
        wk = wqkv[:, HQ * D + g * D:HQ * D + (g + 1) * D]
        wv = wqkv[:, (HQ + HK) * D + g * D:(HQ + HK) * D + (g + 1) * D]
        wqkv_c = np.concatenate([wq, wk, wv], axis=1).astype(bf16)
        vg_c = vg[:, g * D:(g + 1) * D].astype(bf16)
        in_maps.append({
            "HST": hstp,
            "WQKV": np.ascontiguousarray(wqkv_c.reshape(KT, 128, 512)),
            "KGT": np.ascontiguousarray(kg[:, g * D:(g + 1) * D].T.astype(bf16)),
            "VG": np.ascontiguousarray(
                vg_c.reshape(ST, 128, D).transpose(1, 0, 2).reshape(128, -1)),
            "WO": wo,
            "WG": np.ascontiguousarray(wg[:, 2 * c:2 * c + 2].astype(bf16)),
            "BG": np.ascontiguousarray(bg[2 * c:2 * c + 2].reshape(1, 2)),
            "CSF": csf,
            "SNF": snf,
            "ONES": ones,
            "IDN": idn,
            "MASKD": maskd,
            "MASKB": maskb,
        })
    return in_maps


def kernel(**inputs):
    if "nc" not in _CACHE:
        _CACHE["nc"] = _build()
    nc = _CACHE["nc"]
    in_maps = _host_prep(**inputs)
    res = run_bass_kernel_spmd(nc, in_maps, core_ids=list(range(NCORES)))
    out = np.empty((T, HID), np.float32)
    for c in range(NCORES):
        o = res.results[c]["OUT"]
        out[128 * c:128 * (c + 1)] = o[0:128]
        out[1024 + 128 * c:1024 + 128 * (c + 1)] = o[128:256]
    return out


# revision 10
# speedup vs baseline: 1.0260x; 1.0260x over previous
"""Trainium2 Bass kernel for LoopCoderAttention (sparse_attention).

Head-sharded tensor parallelism over 8 NeuronCores:
  core c owns query heads {2c, 2c+1} and KV head c//2.
All on-device tensors live in transposed [feature, token] layout so every
matmul contracts along the partition dim with zero on-device transposes
(except v, which needs one PE transpose per 128-tile).

v4: bf16 matmul operands throughout, causal/band column trimming,
host-pretiled DRAM layouts for contiguous DMA, and a single
software-pipelined attention stream: the QK matmuls run two tiles ahead
of the PV/SM matmuls (hiding the Exp latency), and each head-chunk's
combine/normalize work is deferred into the next head-chunk's tile
stream so it never blocks the PE. gpsimd only runs the partition
broadcasts and collectives. o_proj goes hi-half-first with per-block
output drains so the second all-to-all and output DMA hide behind
matmuls.

o_proj: a 1MB AllToAll (2x 512KB) reshards attention output from
head-sharded to token-sharded; each core then runs the full 2048-deep
contraction for its 256-token slice.
"""
import sys
sys.path.insert(0, '/opt/trn_rl_repo')
import numpy as np
import ml_dtypes
import concourse.bass as bass
import concourse.mybir as mybir
import concourse.tile as tile
from concourse import bacc
from concourse.bass_utils import run_bass_kernel_spmd

T = 2048
HID = 2048
HQ = 16
HK = 4
D = 128
WIN = 64
THETA = 10000.0
SCALE = D ** -0.5
NCORES = 8
TCH = 512                 # t-chunk (matmul free dim)
NCH = T // TCH            # 4 chunks
KT = HID // 128           # 16 k-tiles for 2048-deep contractions
ST = T // 128             # 16 s-tiles
TSL = T // NCORES         # 256-token output slice per core
MASKV = -1e9

F32 = mybir.dt.float32
BF16 = mybir.dt.bfloat16
AF = mybir.ActivationFunctionType

_CACHE = {}


def _build():
    nc = bacc.Bacc("TRN2", target_bir_lowering=False, debug=False,
                   num_devices=NCORES)
    HST = nc.dram_tensor("HST", [KT, NCH, 128, TCH], BF16,
                         kind="ExternalInput").ap()
    WQKV = nc.dram_tensor("WQKV", [KT, 128, 512], BF16,
                          kind="ExternalInput").ap()
    KGT = nc.dram_tensor("KGT", [D, T], BF16, kind="ExternalInput").ap()
    VG = nc.dram_tensor("VG", [128, ST * D], BF16, kind="ExternalInput").ap()
    WO = nc.dram_tensor("WO", [HID, HID], BF16, kind="ExternalInput").ap()
    WG = nc.dram_tensor("WG", [D, 2], BF16, kind="ExternalInput").ap()
    BG = nc.dram_tensor("BG", [1, 2], F32, kind="ExternalInput").ap()
    CSF = nc.dram_tensor("CSF", [128, T], BF16, kind="ExternalInput").ap()
    SNF = nc.dram_tensor("SNF", [128, T], BF16, kind="ExternalInput").ap()
    ONES = nc.dram_tensor("ONES", [128, 1], BF16, kind="ExternalInput").ap()
    IDN = nc.dram_tensor("IDN", [128, 128], BF16, kind="ExternalInput").ap()
    MASKD = nc.dram_tensor("MASKD", [128, 128], F32, kind="ExternalInput").ap()
    MASKB = nc.dram_tensor("MASKB", [128, 192], F32, kind="ExternalInput").ap()
    OUT = nc.dram_tensor("OUT", [TSL, HID], F32, kind="ExternalOutput").ap()

    with tile.TileContext(nc) as tc:
        # pools are a strict stack: creation order is the reverse of the
        # release order at each phase boundary
        const = tc.alloc_tile_pool(name="const", bufs=1)
        dram = tc.alloc_tile_pool(name="dram", bufs=1, space="DRAM")
        aoutp = tc.alloc_tile_pool(name="aoutp", bufs=3)
        opool = tc.alloc_tile_pool(name="opool", bufs=1)
        wop = tc.alloc_tile_pool(name="wop", bufs=1)
        osb = tc.alloc_tile_pool(name="osb", bufs=4)
        work = tc.alloc_tile_pool(name="work", bufs=1)
        ropet = tc.alloc_tile_pool(name="ropet", bufs=2)
        rcpp = tc.alloc_tile_pool(name="rcpp", bufs=2)
        bcp = tc.alloc_tile_pool(name="bcp", bufs=2)
        combp = tc.alloc_tile_pool(name="combp", bufs=3)
        wqkvp = tc.alloc_tile_pool(name="wqkvp", bufs=1)
        chunkp = tc.alloc_tile_pool(name="chunkp", bufs=2)
        hsp = tc.alloc_tile_pool(name="hsp", bufs=16)
        ps1 = tc.alloc_tile_pool(name="ps1", bufs=7, space="PSUM")

        # ---- phase-1 constants first (critical path to first matmul) ----
        wqkv_sb = wqkvp.tile([128, KT, 512], BF16)
        hs_first = []
        n0 = 0
        for k in range(KT):
            nc.sync.dma_start(out=wqkv_sb[:, k, :], in_=WQKV[k])
            hs_t = hsp.tile([128, TCH], BF16, tag="hs_t", name=f"hsf{k}")
            nc.sync.dma_start(out=hs_t[:], in_=HST[k, n0])
            hs_first.append(hs_t)
        csf_sb = wqkvp.tile([128, T], BF16)
        snf_sb = wqkvp.tile([128, T], BF16)
        idn_sb = wqkvp.tile([128, 128], BF16)
        wg_sb = const.tile([D, 2], BF16)
        nc.sync.dma_start(out=wg_sb[:], in_=WG)
        bg_sb = const.tile([1, 2], F32)
        nc.sync.dma_start(out=bg_sb[:], in_=BG)
        # attention-phase constants (scheduler fills DMA idle time)
        kgt_sb = const.tile([D, T], BF16)
        vg_sb = const.tile([128, ST, D], BF16)
        ones_sb = const.tile([128, 1], BF16)
        maskd_sb = const.tile([128, 128], F32)
        maskb_sb = const.tile([128, 192], F32)

        # ---- persistent work tiles (through attention) ----
        qrot = work.tile([128, 2, T], BF16)
        krot = work.tile([128, T], BF16)
        vcur = work.tile([128, ST, D], BF16)   # current v in [s, d] tiles
        gate = work.tile([8, TCH], F32)        # row 2n+h (DMA-staged access)

        a2ai_hi = dram.tile([NCORES, 2 * D, TSL // 2], BF16)
        a2ao_hi = dram.tile([NCORES, 2 * D, TSL // 2], BF16)
        a2ai_lo = dram.tile([NCORES, 2 * D, TSL // 2], BF16)
        a2ao_lo = dram.tile([NCORES, 2 * D, TSL // 2], BF16)

        def rope_chunk(dst_full, src, n):
            """dst_full[:, n*TCH:...] = neox-rope of chunk tile src [128, TCH].

            rot = src * [cos;cos] + rot90(src) * [-sin;sin], where rot90 swaps
            the two 64-partition halves (built with two SBUF->SBUF DMAs since
            DVE ops require matching base partitions).
            """
            sl = bass.ds(n * TCH, TCH)
            sr = ropet.tile([128, TCH], BF16, tag="ropesr", name=f"sr{n}")
            nc.sync.dma_start(out=sr[0:64, :], in_=src[64:128, :])
            nc.sync.dma_start(out=sr[64:128, :], in_=src[0:64, :])
            ta = ropet.tile([128, TCH], BF16, tag="ropetmp", name=f"ra{n}")
            tb = ropet.tile([128, TCH], BF16, tag="ropetmp", name=f"rb{n}")
            nc.vector.tensor_mul(ta[:], src[:], csf_sb[:, sl])
            nc.vector.tensor_mul(tb[:], sr[:], snf_sb[:, sl])
            nc.vector.tensor_add(dst_full[:, sl], ta[:], tb[:])

        # ================= phase 1: qkvT = wqkv^T @ hsT =================
        # chunks ascend so the flush-tail small ops belong to chunk 3 --
        # whose rope/gate results attention needs first and waits on least
        pending_small = []
        for n in range(NCH):
            pss = [ps1.tile([128, TCH], F32, tag="ps1t", name=f"ps1_{n}_{m}")
                   for m in range(4)]
            for k in range(KT):
                if n == n0:
                    hs_t = hs_first[k]
                else:
                    hs_t = hsp.tile([128, TCH], BF16, tag="hs_t",
                                    name=f"hs_{n}_{k}")
                    nc.sync.dma_start(out=hs_t[:], in_=HST[k, n])
                for m in range(4):
                    nc.tensor.matmul(pss[m][:],
                                     wqkv_sb[:, k, m * 128:(m + 1) * 128],
                                     hs_t[:],
                                     start=(k == 0), stop=(k == KT - 1))
            if n == n0:
                # rope tables + identity: after the hot first-chunk DMAs,
                # before their first readers below
                nc.sync.dma_start(out=csf_sb[:], in_=CSF)
                nc.sync.dma_start(out=snf_sb[:], in_=SNF)
                nc.sync.dma_start(out=idn_sb[:], in_=IDN)
            if pending_small:
                pending_small.pop(0)()
            sl = bass.ds(n * TCH, TCH)
            q0c = chunkp.tile([128, TCH], BF16, tag="q0c")
            q1c = chunkp.tile([128, TCH], BF16, tag="q1c")
            kc = chunkp.tile([128, TCH], BF16, tag="kc")
            vc = chunkp.tile([128, TCH], BF16, tag="vc")
            nc.scalar.activation(q0c[:], pss[0][:], AF.Copy)
            nc.scalar.activation(q1c[:], pss[1][:], AF.Copy)
            nc.scalar.activation(kc[:], pss[2][:], AF.Copy)
            nc.vector.tensor_copy(vc[:], pss[3][:])

            rope_chunk(qrot[:, 0, :], q0c, n)
            rope_chunk(qrot[:, 1, :], q1c, n)
            rope_chunk(krot, kc, n)

            def small_ops(n=n, vc=vc, sl=sl):
                # v transposes + gates for chunk n: emitted one chunk later so
                # the PE stream never waits on the DVE rope/copy latency
                for j in range(4):
                    s = 4 * n + j
                    pt = ps1.tile([128, 128], BF16, tag="ps1g",
                                  name=f"pt{s}", bufs=1)
                    nc.tensor.transpose(pt[:], vc[:, j * 128:(j + 1) * 128],
                                        idn_sb[:])
                    nc.vector.tensor_copy(vcur[:, s, :], pt[:])
                for h in range(2):
                    r = 2 * n + h
                    gp = ps1.tile([1, TCH], F32, tag="ps1g",
                                  name=f"gp{r}", bufs=1)
                    nc.tensor.matmul(gp[:], wg_sb[:, h:h + 1], qrot[:, h, sl],
                                     start=True, stop=True)
                    gst = chunkp.tile([1, TCH], F32, tag="gst", name=f"gst{r}")
                    nc.scalar.activation(gst[:], gp[:], AF.Sigmoid,
                                         bias=bg_sb[0:1, h:h + 1])
                    nc.sync.dma_start(out=gate[r:r + 1, :], in_=gst[:])

            pending_small.append(small_ops)

        for f in pending_small:
            f()
        pending_small.clear()

        nc.sync.dma_start(out=kgt_sb[:], in_=KGT)
        nc.sync.dma_start(out=vg_sb[:],
                          in_=VG.rearrange("p (s d) -> p s d", d=D))
        nc.sync.dma_start(out=ones_sb[:], in_=ONES)
        nc.sync.dma_start(out=maskd_sb[:], in_=MASKD)
        nc.sync.dma_start(out=maskb_sb[:], in_=MASKB)

        ps1.release()
        hsp.release()
        chunkp.release()
        wqkvp.release()

        afull_hi = opool.tile([128, KT, TSL // 2], BF16)
        afull_lo = opool.tile([128, KT, TSL // 2], BF16)
        wo_sb = wop.tile([128, KT, HID], BF16)

        expp = tc.alloc_tile_pool(name="expp", bufs=6)
        psqk = tc.alloc_tile_pool(name="psqk", bufs=3, space="PSUM")
        pspv = tc.alloc_tile_pool(name="pspv", bufs=4, space="PSUM")
        pssm = tc.alloc_tile_pool(name="pssm", bufs=1, space="PSUM")

        # ============ phase 2: attention (global + local) ============
        # One flat software-pipelined stream over all 8 head-chunks.
        # Head-chunks descend over token chunks so the high-token half
        # finishes first and its all-to-all overlaps the low-token half.
        iters = []
        for n in reversed(range(NCH)):
            for h in range(2):
                tiles = []
                for s in range(4 * n + 4):
                    j = s - 4 * n
                    if j < 0:
                        tiles.append((kgt_sb, vg_sb, s, 0, TCH, None, 0))
                    else:
                        tiles.append((kgt_sb, vg_sb, s, 128 * j, TCH,
                                      maskd_sb[:], 0))
                ng = len(tiles)
                for j in range(-1, 4):
                    s = 4 * n + j
                    if s < 0:
                        continue
                    x0 = max(0, 128 * j)
                    x1 = min(TCH, 128 * j + 192)
                    y0 = x0 - 128 * j
                    tiles.append((krot, vcur, s, x0, x1,
                                  maskb_sb[:, y0:y0 + (x1 - x0)], 1))
                iters.append({"n": n, "h": h, "r": 2 * n + h,
                              "tiles": tiles, "ng": ng})

        flat = []           # (iter_dict, local_tile_idx)
        for itd in iters:
            for ti in range(len(itd["tiles"])):
                flat.append((itd, ti))
        NT = len(flat)

        def iter_setup(itd):
            """Allocate psum/staging tiles + gate fetch at head-chunk start."""
            r = itd["r"]
            itd["q_ap"] = qrot[:, itd["h"], bass.ds(itd["n"] * TCH, TCH)]
            gg = rcpp.tile([1, 1024], F32, tag="gg", name=f"gg{r}")
            nc.sync.dma_start(out=gg[0:1, 0:TCH], in_=gate[r:r + 1, :])
            nc.vector.tensor_scalar(gg[0:1, TCH:2 * TCH], gg[0:1, 0:TCH],
                                    -1.0, 1.0,
                                    mybir.AluOpType.mult,
                                    mybir.AluOpType.add)
            itd["gg"] = gg
            itd["pv"] = (pspv.tile([128, TCH], F32, tag="pv", name=f"pvg{r}"),
                         pspv.tile([128, TCH], F32, tag="pv", name=f"pvl{r}"))
            itd["sm"] = (pssm.tile([1, TCH], F32, tag="sm", name=f"smg{r}"),
                         pssm.tile([1, TCH], F32, tag="sm", name=f"sml{r}"))
            itd["srec"] = rcpp.tile([1, 1024], F32, tag="srec", name=f"sr{r}")

        def emit_qk(gi):
            itd, ti = flat[gi]
            kT_ap, _, s, x0, x1, _, _ = itd["tiles"][ti]
            if ti == 0:
                iter_setup(itd)
            qk = psqk.tile([128, TCH], F32, tag="qk",
                           name=f"qk{itd['r']}_{ti}")
            nc.tensor.matmul(qk[:, x0:x1], kT_ap[:, s * 128:(s + 1) * 128],
                             itd["q_ap"][:, x0:x1], start=True, stop=True)
            itd.setdefault("qks", {})[ti] = qk

        def emit_combine(itd):
            """Deferred normalize+gate+combine for a finished head-chunk."""
            r, n, h = itd["r"], itd["n"], itd["h"]
            pv_g, pv_l = itd["pv"]
            srec, gg = itd["srec"], itd["gg"]
            rcp = rcpp.tile([1, 1024], F32, tag="rcp", name=f"rc{r}",
                            bufs=1)
            agal = rcpp.tile([1, 1024], F32, tag="agal", name=f"aa{r}",
                             bufs=1)
            nc.vector.reciprocal_approx_fast(rcp[:], srec[:])
            nc.vector.tensor_mul(agal[:], gg[:], rcp[:])
            bg_t = bcp.tile([128, TCH], F32, tag="bcast", name=f"bg{r}")
            bl_t = bcp.tile([128, TCH], F32, tag="bcast", name=f"bl{r}")
            nc.gpsimd.partition_broadcast(bg_t[:], agal[0:1, 0:TCH])
            nc.gpsimd.partition_broadcast(bl_t[:], agal[0:1, TCH:2 * TCH])
            t1 = combp.tile([128, TCH], F32, tag="comb", name=f"t1{r}")
            t2 = combp.tile([128, TCH], F32, tag="comb", name=f"t2{r}")
            ao = aoutp.tile([128, TCH], BF16, tag="aout", name=f"ao{r}")
            nc.vector.tensor_mul(t1[:], pv_g[:], bg_t[:])
            nc.vector.tensor_mul(t2[:], pv_l[:], bl_t[:])
            nc.vector.tensor_add(ao[:], t1[:], t2[:])
            # ship finished 128-col blocks to a2a staging: token 1024+128c
            # (hi) / 128c (lo) lives in chunk n at column offset 128j
            buf = a2ai_hi if n >= 2 else a2ai_lo
            c0 = (n - 2) * 4 if n >= 2 else n * 4
            for j in range(4):
                nc.sync.dma_start(
                    out=buf[c0 + j, h * D:(h + 1) * D, :],
                    in_=ao[:, j * 128:(j + 1) * 128])
            if n == 2 and h == 1:
                # all-to-all #1: high-token halves (overlaps chunks 1,0)
                nc.gpsimd.collective_compute(
                    "AllToAll", mybir.AluOpType.bypass,
                    replica_groups=[list(range(NCORES))],
                    ins=[a2ai_hi[:].opt()], outs=[a2ao_hi[:].opt()])

        LA = 2       # qk matmuls run LA tiles ahead of pv/sm
        DEFER = 3    # previous head-chunk's combine lands after this many
        pending_combine = None
        wo_k = 0
        for gi in range(min(LA, NT)):
            emit_qk(gi)
        for gi in range(NT):
            itd, ti = flat[gi]
            if gi + LA < NT:
                emit_qk(gi + LA)
            if pending_combine is not None and ti == DEFER:
                emit_combine(pending_combine)
                pending_combine = None
                # w_o preload spread across the attention phase
                nk = min(3, KT - wo_k)
                for k in range(wo_k, wo_k + nk):
                    nc.sync.dma_start(out=wo_sb[:, k, :],
                                      in_=WO[k * 128:(k + 1) * 128, :])
                wo_k += nk
            _, v_ap, s, x0, x1, m_ap, ch = itd["tiles"][ti]
            qk = itd["qks"].pop(ti)
            if m_ap is not None:
                mw = m_ap.shape[-1]
                nc.vector.tensor_add(qk[:, x0:x0 + mw],
                                     qk[:, x0:x0 + mw], m_ap)
            ex = expp.tile([128, TCH], BF16, tag="ex",
                           name=f"ex{itd['r']}_{ti}")
            nc.scalar.activation(ex[:, x0:x1], qk[:, x0:x1],
                                 AF.Exp, scale=SCALE)
            ng, nt = itd["ng"], len(itd["tiles"])
            first = (ti == 0) or (ti == ng)
            last = (ti == ng - 1) or (ti == nt - 1)
            nc.tensor.matmul(itd["pv"][ch][:, x0:x1], v_ap[:, s, :],
                             ex[:, x0:x1], start=first, stop=last,
                             skip_group_check=True)
            nc.tensor.matmul(itd["sm"][ch][0:1, x0:x1], ones_sb[:],
                             ex[:, x0:x1], start=first, stop=last,
                             skip_group_check=True)
            if ti == ng - 1:
                # drain the global sum right away so its psum slot can
                # rotate to the local sum (pssm bufs=1)
                nc.scalar.activation(itd["srec"][0:1, 0:TCH],
                                     itd["sm"][0][:], AF.Copy)
            if ti == nt - 1:
                nc.scalar.activation(itd["srec"][0:1, TCH:2 * TCH],
                                     itd["sm"][1][:], AF.Copy)
                if itd["n"] == 2 and itd["h"] == 1:
                    # this combine gates all-to-all #1: emit immediately so
                    # the collective launches while chunks 1,0 still compute
                    emit_combine(itd)
                else:
                    pending_combine = itd
        emit_combine(pending_combine)

        pssm.release()
        pspv.release()
        psqk.release()
        expp.release()

        # ========= phase 3: all-to-all #2 (low-token halves) =========
        nc.gpsimd.collective_compute(
            "AllToAll", mybir.AluOpType.bypass,
            replica_groups=[list(range(NCORES))],
            ins=[a2ai_lo[:].opt()], outs=[a2ao_lo[:].opt()])

        combp.release()
        bcp.release()
        rcpp.release()
        ropet.release()
        work.release()

        pso = tc.alloc_tile_pool(name="pso", bufs=8, space="PSUM")

        a2ao_hi_v = (a2ao_hi[:].rearrange("c p n -> (c p) n")
                     .rearrange("(k p) n -> p k n", p=128))
        a2ao_lo_v = (a2ao_lo[:].rearrange("c p n -> (c p) n")
                     .rearrange("(k p) n -> p k n", p=128))
        for k in range(KT):
            nc.sync.dma_start(out=afull_hi[:, k, :], in_=a2ao_hi_v[:, k, :])
        for k in range(KT):
            nc.sync.dma_start(out=afull_lo[:, k, :], in_=a2ao_lo_v[:, k, :])

        # ============ phase 4: o_proj for our token slice ============
        # hi half first: its matmuls only need all-to-all #1, so they hide
        # all-to-all #2's latency; per-block drains overlap the output DMA
        # with the remaining matmuls; OUT rows 0-127 = low, 128-255 = high
        for tt, afull in ((1, afull_hi), (0, afull_lo)):
            for e in range(NCH):
                po = pso.tile([128, TCH], F32, tag="po", name=f"po_{tt}_{e}")
                for k in range(KT):
                    nc.tensor.matmul(po[:],
                                     afull[:, k, :],
                                     wo_sb[:, k, e * TCH:(e + 1) * TCH],
                                     start=(k == 0), stop=(k == KT - 1))
                ot = osb.tile([128, TCH], F32, tag="ot", name=f"ot{tt}_{e}")
                nc.scalar.activation(ot[:], po[:], AF.Copy)
                nc.sync.dma_start(
                    out=OUT[tt * 128:(tt + 1) * 128,
                            e * TCH:(e + 1) * TCH],
                    in_=ot[:])
        pso.release()
        osb.release()
        wop.release()
        opool.release()
        aoutp.release()
        dram.release()
        const.release()

    nc.compile()
    return nc


def _host_prep(hidden_states, positions, k_global, v_global, w_qkv, w_o,
               w_gate, b_gate):
    """Layout-only host transforms + constant tables -> per-core in_maps."""
    f32 = np.float32
    bf16 = ml_dtypes.bfloat16
    hs = np.asarray(hidden_states, f32)
    pos = np.asarray(positions)
    kg = np.asarray(k_global, f32)
    vg = np.asarray(v_global, f32)
    wqkv = np.asarray(w_qkv, f32)
    wo = np.ascontiguousarray(np.asarray(w_o, f32).astype(bf16))
    wg = np.asarray(w_gate, f32)
    bg = np.asarray(b_gate, f32)

    # [feature, token] transposed activations, pretiled [k, n, 128, 512]
    # so every on-device tile is one contiguous DMA
    hst = hs.T.astype(bf16)
    hstp = np.ascontiguousarray(
        hst.reshape(KT, 128, NCH, TCH).transpose(0, 2, 1, 3))

    half = D // 2
    inv_freq = (THETA ** (-np.arange(half, dtype=f32) / half)).astype(f32)
    ang = pos.astype(f32)[:, None] * inv_freq[None, :]
    cos_t = np.cos(ang).astype(f32).T       # [64, T]
    sin_t = np.sin(ang).astype(f32).T
    csf = np.ascontiguousarray(
        np.concatenate([cos_t, cos_t], axis=0).astype(bf16))
    snf = np.ascontiguousarray(
        np.concatenate([-sin_t, sin_t], axis=0).astype(bf16))

    p = np.arange(128, dtype=np.int64)[:, None]
    # diagonal causal block: valid iff col >= partition
    xr = np.arange(128, dtype=np.int64)[None, :]
    maskd = np.where(xr >= p, 0.0, MASKV).astype(f32)
    # sliding-window band: valid iff 0 <= y - p <= WIN
    y = np.arange(192, dtype=np.int64)[None, :]
    maskb = np.where((y - p >= 0) & (y - p <= WIN), 0.0, MASKV).astype(f32)

    ones = np.ones((128, 1), bf16)
    idn = np.eye(128, dtype=bf16)

    in_maps = []
    for c in range(NCORES):
        g = c // 2
        wq = wqkv[:, 2 * c * D:(2 * c + 2) * D]
        wk = wqkv[:, HQ * D + g * D:HQ * D + (g + 1) * D]
        wv = wqkv[:, (HQ + HK) * D + g * D:(HQ + HK) * D + (g + 1) * D]
        wqkv_c = np.concatenate([wq, wk, wv], axis=1).astype(bf16)
        vg_c = vg[:, g * D:(g + 1) * D].astype(bf16)
        in_maps.append({
            "HST": hstp,
            "WQKV": np.ascontiguousarray(wqkv_c.reshape(KT, 128, 512)),
            "KGT": np.ascontiguousarray(kg[:, g * D:(g + 1) * D].T.astype(bf16)),
            "VG": np.ascontiguousarray(
                vg_c.reshape(ST, 128, D).transpose(1, 0, 2).reshape(128, -1)),
            "WO": wo,
            "WG": np.ascontiguousarray(wg[:, 2 * c:2 * c + 2].astype(bf16)),
            "BG": np.ascontiguousarray(bg[2 * c:2 * c + 2].reshape(1, 2)),
            "CSF": csf,
            "SNF": snf,
            "ONES": ones,
            "IDN": idn,
            "MASKD": maskd,
            "MASKB": maskb,
        })
    return in_maps


def kernel(**inputs):
    if "nc" not in _CACHE:
        _CACHE["nc"] = _build()
    nc = _CACHE["nc"]
    in_maps = _host_prep(**inputs)
    res = run_bass_kernel_spmd(nc, in_maps, core_ids=list(range(NCORES)))
    out = np.empty((T, HID), np.float32)
    for c in range(NCORES):
        o = res.results[c]["OUT"]
        out[128 * c:128 * (c + 1)] = o[0:128]
        out[1024 + 128 * c:1024 + 128 * (c + 1)] = o[128:256]
    return out


# revision 11
# speedup vs baseline: 1.0429x; 1.0165x over previous
"""Trainium2 Bass kernel for LoopCoderAttention (sparse_attention).

Head-sharded tensor parallelism over 8 NeuronCores:
  core c owns query heads {2c, 2c+1} and KV head c//2.
All on-device tensors live in transposed [feature, token] layout so every
matmul contracts along the partition dim with zero on-device transposes
(except v, which needs one PE transpose per 128-tile).

v4: bf16 matmul operands throughout, causal/band column trimming,
host-pretiled DRAM layouts for contiguous DMA, and a single
software-pipelined attention stream: the QK matmuls run two tiles ahead
of the PV/SM matmuls (hiding the Exp latency), and each head-chunk's
combine/normalize work is deferred into the next head-chunk's tile
stream so it never blocks the PE. gpsimd only runs the partition
broadcasts and collectives. o_proj goes hi-half-first with per-block
output drains so the second all-to-all and output DMA hide behind
matmuls.

o_proj: a 1MB AllToAll (2x 512KB) reshards attention output from
head-sharded to token-sharded; each core then runs the full 2048-deep
contraction for its 256-token slice.
"""
import sys
sys.path.insert(0, '/opt/trn_rl_repo')
import numpy as np
import ml_dtypes
import concourse.bass as bass
import concourse.mybir as mybir
import concourse.tile as tile
from concourse import bacc
from concourse.bass_utils import run_bass_kernel_spmd

T = 2048
HID = 2048
HQ = 16
HK = 4
D = 128
WIN = 64
THETA = 10000.0
SCALE = D ** -0.5
NCORES = 8
TCH = 512                 # t-chunk (matmul free dim)
NCH = T // TCH            # 4 chunks
KT = HID // 128           # 16 k-tiles for 2048-deep contractions
ST = T // 128             # 16 s-tiles
TSL = T // NCORES         # 256-token output slice per core
MASKV = -1e9

F32 = mybir.dt.float32
BF16 = mybir.dt.bfloat16
AF = mybir.ActivationFunctionType

_CACHE = {}


def _build():
    nc = bacc.Bacc("TRN2", target_bir_lowering=False, debug=False,
                   num_devices=NCORES)
    HST = nc.dram_tensor("HST", [KT, NCH, 128, TCH], BF16,
                         kind="ExternalInput").ap()
    WQKV = nc.dram_tensor("WQKV", [KT, 128, 512], BF16,
                          kind="ExternalInput").ap()
    KGT = nc.dram_tensor("KGT", [D, T], BF16, kind="ExternalInput").ap()
    VG = nc.dram_tensor("VG", [128, ST * D], BF16, kind="ExternalInput").ap()
    WO = nc.dram_tensor("WO", [HID, HID], BF16, kind="ExternalInput").ap()
    WG = nc.dram_tensor("WG", [D, 2], BF16, kind="ExternalInput").ap()
    BG = nc.dram_tensor("BG", [1, 2], F32, kind="ExternalInput").ap()
    CSF = nc.dram_tensor("CSF", [128, T], BF16, kind="ExternalInput").ap()
    SNF = nc.dram_tensor("SNF", [128, T], BF16, kind="ExternalInput").ap()
    ONES = nc.dram_tensor("ONES", [128, 1], BF16, kind="ExternalInput").ap()
    IDN = nc.dram_tensor("IDN", [128, 128], BF16, kind="ExternalInput").ap()
    MASKD = nc.dram_tensor("MASKD", [128, 128], F32, kind="ExternalInput").ap()
    MASKB = nc.dram_tensor("MASKB", [128, 192], F32, kind="ExternalInput").ap()
    OUT = nc.dram_tensor("OUT", [TSL, HID], F32, kind="ExternalOutput").ap()

    with tile.TileContext(nc) as tc:
        # pools are a strict stack: creation order is the reverse of the
        # release order at each phase boundary
        const = tc.alloc_tile_pool(name="const", bufs=1)
        dram = tc.alloc_tile_pool(name="dram", bufs=1, space="DRAM")
        aoutp = tc.alloc_tile_pool(name="aoutp", bufs=3)
        opool = tc.alloc_tile_pool(name="opool", bufs=1)
        wop = tc.alloc_tile_pool(name="wop", bufs=1)
        osb = tc.alloc_tile_pool(name="osb", bufs=4)
        work = tc.alloc_tile_pool(name="work", bufs=1)
        ropet = tc.alloc_tile_pool(name="ropet", bufs=2)
        rcpp = tc.alloc_tile_pool(name="rcpp", bufs=2)
        bcp = tc.alloc_tile_pool(name="bcp", bufs=2)
        combp = tc.alloc_tile_pool(name="combp", bufs=3)
        wqkvp = tc.alloc_tile_pool(name="wqkvp", bufs=1)
        chunkp = tc.alloc_tile_pool(name="chunkp", bufs=2)
        hsp = tc.alloc_tile_pool(name="hsp", bufs=16)
        ps1 = tc.alloc_tile_pool(name="ps1", bufs=7, space="PSUM")

        # ---- phase-1 constants first (critical path to first matmul) ----
        wqkv_sb = wqkvp.tile([128, KT, 512], BF16)
        hs_first = []
        n0 = 0
        for k in range(KT):
            nc.sync.dma_start(out=wqkv_sb[:, k, :], in_=WQKV[k])
            hs_t = hsp.tile([128, TCH], BF16, tag="hs_t", name=f"hsf{k}")
            nc.sync.dma_start(out=hs_t[:], in_=HST[k, n0])
            hs_first.append(hs_t)
        csf_sb = wqkvp.tile([128, T], BF16)
        snf_sb = wqkvp.tile([128, T], BF16)
        idn_sb = wqkvp.tile([128, 128], BF16)
        wg_sb = const.tile([D, 2], BF16)
        nc.sync.dma_start(out=wg_sb[:], in_=WG)
        bg_sb = const.tile([1, 2], F32)
        nc.sync.dma_start(out=bg_sb[:], in_=BG)
        # attention-phase constants (scheduler fills DMA idle time)
        kgt_sb = const.tile([D, T], BF16)
        vg_sb = const.tile([128, ST, D], BF16)
        ones_sb = const.tile([128, 1], BF16)
        maskd_sb = const.tile([128, 128], F32)
        maskb_sb = const.tile([128, 192], F32)

        # ---- persistent work tiles (through attention) ----
        qrot = work.tile([128, 2, T], BF16)
        krot = work.tile([128, T], BF16)
        vcur = work.tile([128, ST, D], BF16)   # current v in [s, d] tiles
        gate = work.tile([8, TCH], F32)        # row 2n+h (DMA-staged access)

        a2ai_hi = dram.tile([NCORES, 2 * D, TSL // 2], BF16)
        a2ao_hi = dram.tile([NCORES, 2 * D, TSL // 2], BF16)
        a2ai_lo = dram.tile([NCORES, 2 * D, TSL // 2], BF16)
        a2ao_lo = dram.tile([NCORES, 2 * D, TSL // 2], BF16)

        def rope_chunk(dst_full, src, n):
            """dst_full[:, n*TCH:...] = neox-rope of chunk tile src [128, TCH].

            rot = src * [cos;cos] + rot90(src) * [-sin;sin], where rot90 swaps
            the two 64-partition halves (built with two SBUF->SBUF DMAs since
            DVE ops require matching base partitions).
            """
            sl = bass.ds(n * TCH, TCH)
            sr = ropet.tile([128, TCH], BF16, tag="ropesr", name=f"sr{n}")
            nc.sync.dma_start(out=sr[0:64, :], in_=src[64:128, :])
            nc.sync.dma_start(out=sr[64:128, :], in_=src[0:64, :])
            ta = ropet.tile([128, TCH], BF16, tag="ropetmp", name=f"ra{n}")
            tb = ropet.tile([128, TCH], BF16, tag="ropetmp", name=f"rb{n}")
            nc.vector.tensor_mul(ta[:], src[:], csf_sb[:, sl])
            nc.vector.tensor_mul(tb[:], sr[:], snf_sb[:, sl])
            nc.vector.tensor_add(dst_full[:, sl], ta[:], tb[:])

        # ================= phase 1: qkvT = wqkv^T @ hsT =================
        # chunks ascend so the flush-tail small ops belong to chunk 3 --
        # whose rope/gate results attention needs first and waits on least
        pending_small = []
        for n in range(NCH):
            pss = [ps1.tile([128, TCH], F32, tag="ps1t", name=f"ps1_{n}_{m}")
                   for m in range(4)]
            for k in range(KT):
                if n == n0:
                    hs_t = hs_first[k]
                else:
                    hs_t = hsp.tile([128, TCH], BF16, tag="hs_t",
                                    name=f"hs_{n}_{k}")
                    nc.sync.dma_start(out=hs_t[:], in_=HST[k, n])
                for m in range(4):
                    nc.tensor.matmul(pss[m][:],
                                     wqkv_sb[:, k, m * 128:(m + 1) * 128],
                                     hs_t[:],
                                     start=(k == 0), stop=(k == KT - 1))
            if n == n0:
                # rope tables + identity: after the hot first-chunk DMAs,
                # before their first readers below
                nc.sync.dma_start(out=csf_sb[:], in_=CSF)
                nc.sync.dma_start(out=snf_sb[:], in_=SNF)
                nc.sync.dma_start(out=idn_sb[:], in_=IDN)
            if pending_small:
                pending_small.pop(0)()
            sl = bass.ds(n * TCH, TCH)
            q0c = chunkp.tile([128, TCH], BF16, tag="q0c")
            q1c = chunkp.tile([128, TCH], BF16, tag="q1c")
            kc = chunkp.tile([128, TCH], BF16, tag="kc")
            vc = chunkp.tile([128, TCH], BF16, tag="vc")
            nc.scalar.activation(q0c[:], pss[0][:], AF.Copy)
            nc.scalar.activation(q1c[:], pss[1][:], AF.Copy)
            nc.scalar.activation(kc[:], pss[2][:], AF.Copy)
            nc.vector.tensor_copy(vc[:], pss[3][:])

            rope_chunk(qrot[:, 0, :], q0c, n)
            rope_chunk(qrot[:, 1, :], q1c, n)
            rope_chunk(krot, kc, n)

            def small_ops(n=n, vc=vc, sl=sl):
                # v transposes + gates for chunk n: emitted one chunk later so
                # the PE stream never waits on the DVE rope/copy latency
                for j in range(4):
                    s = 4 * n + j
                    pt = ps1.tile([128, 128], BF16, tag="ps1g",
                                  name=f"pt{s}", bufs=1)
                    nc.tensor.transpose(pt[:], vc[:, j * 128:(j + 1) * 128],
                                        idn_sb[:])
                    nc.vector.tensor_copy(vcur[:, s, :], pt[:])
                for h in range(2):
                    r = 2 * n + h
                    gp = ps1.tile([1, TCH], F32, tag="ps1g",
                                  name=f"gp{r}", bufs=1)
                    nc.tensor.matmul(gp[:], wg_sb[:, h:h + 1], qrot[:, h, sl],
                                     start=True, stop=True)
                    gst = chunkp.tile([1, TCH], F32, tag="gst", name=f"gst{r}")
                    nc.scalar.activation(gst[:], gp[:], AF.Sigmoid,
                                         bias=bg_sb[0:1, h:h + 1])
                    nc.sync.dma_start(out=gate[r:r + 1, :], in_=gst[:])

            pending_small.append(small_ops)

        for f in pending_small:
            f()
        pending_small.clear()

        nc.sync.dma_start(out=kgt_sb[:], in_=KGT)
        nc.sync.dma_start(out=vg_sb[:],
                          in_=VG.rearrange("p (s d) -> p s d", d=D))
        nc.sync.dma_start(out=ones_sb[:], in_=ONES)
        nc.sync.dma_start(out=maskd_sb[:], in_=MASKD)
        nc.sync.dma_start(out=maskb_sb[:], in_=MASKB)

        ps1.release()
        hsp.release()
        chunkp.release()
        wqkvp.release()

        afull_hi = opool.tile([128, KT, TSL // 2], BF16)
        afull_lo = opool.tile([128, KT, TSL // 2], BF16)
        wo_sb = wop.tile([128, KT, HID], BF16)

        expp = tc.alloc_tile_pool(name="expp", bufs=6)
        psqk = tc.alloc_tile_pool(name="psqk", bufs=3, space="PSUM")
        pspv = tc.alloc_tile_pool(name="pspv", bufs=4, space="PSUM")
        pssm = tc.alloc_tile_pool(name="pssm", bufs=1, space="PSUM")

        # ============ phase 2: attention (global + local) ============
        # One flat software-pipelined stream over all 8 head-chunks.
        # Head-chunks descend over token chunks so the high-token half
        # finishes first and its all-to-all overlaps the low-token half.
        iters = []
        for n in reversed(range(NCH)):
            for h in range(2):
                tiles = []
                for s in range(4 * n + 4):
                    j = s - 4 * n
                    if j < 0:
                        tiles.append((kgt_sb, vg_sb, s, 0, TCH, None, 0))
                    else:
                        tiles.append((kgt_sb, vg_sb, s, 128 * j, TCH,
                                      maskd_sb[:], 0))
                ng = len(tiles)
                for j in range(-1, 4):
                    s = 4 * n + j
                    if s < 0:
                        continue
                    x0 = max(0, 128 * j)
                    x1 = min(TCH, 128 * j + 192)
                    y0 = x0 - 128 * j
                    tiles.append((krot, vcur, s, x0, x1,
                                  maskb_sb[:, y0:y0 + (x1 - x0)], 1))
                iters.append({"n": n, "h": h, "r": 2 * n + h,
                              "tiles": tiles, "ng": ng})

        flat = []           # (iter_dict, local_tile_idx)
        for itd in iters:
            for ti in range(len(itd["tiles"])):
                flat.append((itd, ti))
        NT = len(flat)

        def setup_gg(itd):
            # gate fetch one head-chunk ahead: the tiny SBUF-SBUF DMA must
            # not sit behind bulk w_o/staging traffic on the DVE's critical
            # path when its head-chunk starts
            r = itd["r"]
            gg = rcpp.tile([1, 1024], F32, tag="gg", name=f"gg{r}", bufs=3)
            nc.sync.dma_start(out=gg[0:1, 0:TCH], in_=gate[r:r + 1, :])
            nc.vector.tensor_scalar(gg[0:1, TCH:2 * TCH], gg[0:1, 0:TCH],
                                    -1.0, 1.0,
                                    mybir.AluOpType.mult,
                                    mybir.AluOpType.add)
            itd["gg"] = gg

        def iter_setup(itd):
            """Allocate psum tiles at head-chunk start."""
            r = itd["r"]
            itd["q_ap"] = qrot[:, itd["h"], bass.ds(itd["n"] * TCH, TCH)]
            itd["pv"] = (pspv.tile([128, TCH], F32, tag="pv", name=f"pvg{r}"),
                         pspv.tile([128, TCH], F32, tag="pv", name=f"pvl{r}"))
            itd["sm"] = (pssm.tile([1, TCH], F32, tag="sm", name=f"smg{r}"),
                         pssm.tile([1, TCH], F32, tag="sm", name=f"sml{r}"))
            itd["srec"] = rcpp.tile([1, 1024], F32, tag="srec", name=f"sr{r}")

        def emit_qk(gi):
            itd, ti = flat[gi]
            kT_ap, _, s, x0, x1, _, _ = itd["tiles"][ti]
            if ti == 0:
                iter_setup(itd)
                ii = iters.index(itd)
                if ii == 0:
                    setup_gg(iters[0])
                    setup_gg(iters[1])
                elif ii + 1 < len(iters):
                    setup_gg(iters[ii + 1])
            qk = psqk.tile([128, TCH], F32, tag="qk",
                           name=f"qk{itd['r']}_{ti}")
            nc.tensor.matmul(qk[:, x0:x1], kT_ap[:, s * 128:(s + 1) * 128],
                             itd["q_ap"][:, x0:x1], start=True, stop=True)
            itd.setdefault("qks", {})[ti] = qk

        def emit_combine(itd):
            """Deferred normalize+gate+combine for a finished head-chunk."""
            r, n, h = itd["r"], itd["n"], itd["h"]
            pv_g, pv_l = itd["pv"]
            srec, gg = itd["srec"], itd["gg"]
            rcp = rcpp.tile([1, 1024], F32, tag="rcp", name=f"rc{r}",
                            bufs=1)
            agal = rcpp.tile([1, 1024], F32, tag="agal", name=f"aa{r}",
                             bufs=1)
            nc.vector.reciprocal_approx_fast(rcp[:], srec[:])
            nc.vector.tensor_mul(agal[:], gg[:], rcp[:])
            bg_t = bcp.tile([128, TCH], F32, tag="bcast", name=f"bg{r}")
            bl_t = bcp.tile([128, TCH], F32, tag="bcast", name=f"bl{r}")
            nc.gpsimd.partition_broadcast(bg_t[:], agal[0:1, 0:TCH])
            nc.gpsimd.partition_broadcast(bl_t[:], agal[0:1, TCH:2 * TCH])
            t1 = combp.tile([128, TCH], F32, tag="comb", name=f"t1{r}")
            t2 = combp.tile([128, TCH], F32, tag="comb", name=f"t2{r}")
            ao = aoutp.tile([128, TCH], BF16, tag="aout", name=f"ao{r}")
            nc.vector.tensor_mul(t1[:], pv_g[:], bg_t[:])
            nc.vector.tensor_mul(t2[:], pv_l[:], bl_t[:])
            nc.vector.tensor_add(ao[:], t1[:], t2[:])
            # ship finished 128-col blocks to a2a staging: token 1024+128c
            # (hi) / 128c (lo) lives in chunk n at column offset 128j
            buf = a2ai_hi if n >= 2 else a2ai_lo
            c0 = (n - 2) * 4 if n >= 2 else n * 4
            for j in range(4):
                nc.sync.dma_start(
                    out=buf[c0 + j, h * D:(h + 1) * D, :],
                    in_=ao[:, j * 128:(j + 1) * 128])
            if n == 2 and h == 1:
                # all-to-all #1: high-token halves (overlaps chunks 1,0)
                nc.gpsimd.collective_compute(
                    "AllToAll", mybir.AluOpType.bypass,
                    replica_groups=[list(range(NCORES))],
                    ins=[a2ai_hi[:].opt()], outs=[a2ao_hi[:].opt()])

        LA = 2       # qk matmuls run LA tiles ahead of pv/sm
        DEFER = 3    # previous head-chunk's combine lands after this many
        pending_combine = None
        wo_k = 0
        for gi in range(min(LA, NT)):
            emit_qk(gi)
        for gi in range(NT):
            itd, ti = flat[gi]
            if gi + LA < NT:
                emit_qk(gi + LA)
            if pending_combine is not None and ti == DEFER:
                emit_combine(pending_combine)
                pending_combine = None
                # w_o preload spread across the attention phase
                nk = min(3, KT - wo_k)
                for k in range(wo_k, wo_k + nk):
                    nc.sync.dma_start(out=wo_sb[:, k, :],
                                      in_=WO[k * 128:(k + 1) * 128, :])
                wo_k += nk
            _, v_ap, s, x0, x1, m_ap, ch = itd["tiles"][ti]
            qk = itd["qks"].pop(ti)
            if m_ap is not None:
                mw = m_ap.shape[-1]
                nc.vector.tensor_add(qk[:, x0:x0 + mw],
                                     qk[:, x0:x0 + mw], m_ap)
            ex = expp.tile([128, TCH], BF16, tag="ex",
                           name=f"ex{itd['r']}_{ti}")
            nc.scalar.activation(ex[:, x0:x1], qk[:, x0:x1],
                                 AF.Exp, scale=SCALE)
            ng, nt = itd["ng"], len(itd["tiles"])
            first = (ti == 0) or (ti == ng)
            last = (ti == ng - 1) or (ti == nt - 1)
            nc.tensor.matmul(itd["pv"][ch][:, x0:x1], v_ap[:, s, :],
                             ex[:, x0:x1], start=first, stop=last,
                             skip_group_check=True)
            nc.tensor.matmul(itd["sm"][ch][0:1, x0:x1], ones_sb[:],
                             ex[:, x0:x1], start=first, stop=last,
                             skip_group_check=True)
            if ti == ng - 1:
                # drain the global sum right away so its psum slot can
                # rotate to the local sum (pssm bufs=1)
                nc.scalar.activation(itd["srec"][0:1, 0:TCH],
                                     itd["sm"][0][:], AF.Copy)
            if ti == nt - 1:
                nc.scalar.activation(itd["srec"][0:1, TCH:2 * TCH],
                                     itd["sm"][1][:], AF.Copy)
                if itd["n"] == 2 and itd["h"] == 1:
                    # this combine gates all-to-all #1: emit immediately so
                    # the collective launches while chunks 1,0 still compute
                    emit_combine(itd)
                else:
                    pending_combine = itd
        emit_combine(pending_combine)

        pssm.release()
        pspv.release()
        psqk.release()
        expp.release()

        # ========= phase 3: all-to-all #2 (low-token halves) =========
        nc.gpsimd.collective_compute(
            "AllToAll", mybir.AluOpType.bypass,
            replica_groups=[list(range(NCORES))],
            ins=[a2ai_lo[:].opt()], outs=[a2ao_lo[:].opt()])

        combp.release()
        bcp.release()
        rcpp.release()
        ropet.release()
        work.release()

        pso = tc.alloc_tile_pool(name="pso", bufs=8, space="PSUM")

        a2ao_hi_v = (a2ao_hi[:].rearrange("c p n -> (c p) n")
                     .rearrange("(k p) n -> p k n", p=128))
        a2ao_lo_v = (a2ao_lo[:].rearrange("c p n -> (c p) n")
                     .rearrange("(k p) n -> p k n", p=128))
        for k in range(KT):
            nc.sync.dma_start(out=afull_hi[:, k, :], in_=a2ao_hi_v[:, k, :])
        for k in range(KT):
            nc.sync.dma_start(out=afull_lo[:, k, :], in_=a2ao_lo_v[:, k, :])

        # ============ phase 4: o_proj for our token slice ============
        # hi half first: its matmuls only need all-to-all #1, so they hide
        # all-to-all #2's latency; per-block drains overlap the output DMA
        # with the remaining matmuls; OUT rows 0-127 = low, 128-255 = high
        for tt, afull in ((1, afull_hi), (0, afull_lo)):
            for e in range(NCH):
                po = pso.tile([128, TCH], F32, tag="po", name=f"po_{tt}_{e}")
                for k in range(KT):
                    nc.tensor.matmul(po[:],
                                     afull[:, k, :],
                                     wo_sb[:, k, e * TCH:(e + 1) * TCH],
                                     start=(k == 0), stop=(k == KT - 1))
                ot = osb.tile([128, TCH], F32, tag="ot", name=f"ot{tt}_{e}")
                nc.scalar.activation(ot[:], po[:], AF.Copy)
                nc.sync.dma_start(
                    out=OUT[tt * 128:(tt + 1) * 128,
                            e * TCH:(e + 1) * TCH],
                    in_=ot[:])
        pso.release()
        osb.release()
        wop.release()
        opool.release()
        aoutp.release()
        dram.release()
        const.release()

    nc.compile()
    return nc


def _host_prep(hidden_states, positions, k_global, v_global, w_qkv, w_o,
               w_gate, b_gate):
    """Layout-only host transforms + constant tables -> per-core in_maps."""
    f32 = np.float32
    bf16 = ml_dtypes.bfloat16
    hs = np.asarray(hidden_states, f32)
    pos = np.asarray(positions)
    kg = np.asarray(k_global, f32)
    vg = np.asarray(v_global, f32)
    wqkv = np.asarray(w_qkv, f32)
    wo = np.ascontiguousarray(np.asarray(w_o, f32).astype(bf16))
    wg = np.asarray(w_gate, f32)
    bg = np.asarray(b_gate, f32)

    # [feature, token] transposed activations, pretiled [k, n, 128, 512]
    # so every on-device tile is one contiguous DMA
    hst = hs.T.astype(bf16)
    hstp = np.ascontiguousarray(
        hst.reshape(KT, 128, NCH, TCH).transpose(0, 2, 1, 3))

    half = D // 2
    inv_freq = (THETA ** (-np.arange(half, dtype=f32) / half)).astype(f32)
    ang = pos.astype(f32)[:, None] * inv_freq[None, :]
    cos_t = np.cos(ang).astype(f32).T       # [64, T]
    sin_t = np.sin(ang).astype(f32).T
    csf = np.ascontiguousarray(
        np.concatenate([cos_t, cos_t], axis=0).astype(bf16))
    snf = np.ascontiguousarray(
        np.concatenate([-sin_t, sin_t], axis=0).astype(bf16))

    p = np.arange(128, dtype=np.int64)[:, None]
    # diagonal causal block: valid iff col >= partition
    xr = np.arange(128, dtype=np.int64)[None, :]
    maskd = np.where(xr >= p, 0.0, MASKV).astype(f32)
    # sliding-window band: valid iff 0 <= y - p <= WIN
    y = np.arange(192, dtype=np.int64)[None, :]
    maskb = np.where((y - p >= 0) & (y - p <= WIN), 0.0, MASKV).astype(f32)

    ones = np.ones((128, 1), bf16)
    idn = np.eye(128, dtype=bf16)

    in_maps = []
    for c in range(NCORES):
        g = c // 2
        wq = wqkv[:, 2 * c * D:(2 * c + 2) * D]
        wk = wqkv[:, HQ * D + g * D:HQ * D + (g + 1) * D]
        wv = wqkv[:, (HQ + HK) * D + g * D:(HQ + HK) * D + (g + 1) * D]
        wqkv_c = np.concatenate([wq, wk, wv], axis=1).astype(bf16)
        vg_c = vg[:, g * D:(g + 1) * D].astype(bf16)
        in_maps.append({
            "HST": hstp,
            "WQKV": np.ascontiguousarray(wqkv_c.reshape(KT, 128, 512)),
            "KGT": np.ascontiguousarray(kg[:, g * D:(g + 1) * D].T.astype(bf16)),
            "VG": np.ascontiguousarray(
                vg_c.reshape(ST, 128, D).transpose(1, 0, 2).reshape(128, -1)),
            "WO": wo,
            "WG": np.ascontiguousarray(wg[:, 2 * c:2 * c + 2].astype(bf16)),
            "BG": np.ascontiguousarray(bg[2 * c:2 * c + 2].reshape(1, 2)),
            "CSF": csf,
            "SNF": snf,
            "ONES": ones,
            "IDN": idn,
            "MASKD": maskd,
            "MASKB": maskb,
        })
    return in_maps


def kernel(**inputs):
    if "nc" not in _CACHE:
        _CACHE["nc"] = _build()
    nc = _CACHE["nc"]
    in_maps = _host_prep(**inputs)
    res = run_bass_kernel_spmd(nc, in_maps, core_ids=list(range(NCORES)))
    out = np.empty((T, HID), np.float32)
    for c in range(NCORES):
        o = res.results[c]["OUT"]
        out[128 * c:128 * (c + 1)] = o[0:128]
        out[1024 + 128 * c:1024 + 128 * (c + 1)] = o[128:256]
    return out
